# revision 39
# baseline (speedup 1.0000x reference)
"""GQA sliding-window attention (training path, no causal mask, no 1/sqrt(d)
scaling) on 8 Trainium2 NeuronCores.

Reference semantics (see original nn.Module):
  q = x@Wq+bq [b,s,16,64]; k,v = x@Wk+bk / x@Wv+bv [b,s,2,64]
  k,v zero-padded by 128 on both sides of s; query i attends padded
  positions [i, i+256) (i.e. global [i-128, i+128)); padded positions
  contribute score 0 (exp->1) and value 0. out = attn @ Wo + bo.

Sharding: batch x sequence. 8 shards = 2 batches x 4 chunks of 512 query
rows. Each core receives x^T for its 512 rows plus a 128-row halo on each
side (zero rows outside [0, 2048)), with an appended 0/1 validity row so
that K/V bias is only added at in-range positions (k = x@Wk + valid*bk).
Host gathers/concatenates per-core outputs; no collectives.

Per-core dataflow (bf16 matmul inputs, fp32 PSUM accumulation):
  - Inputs land bf16 and split in partition-halves across both HWDGE
    rings so many DMA engines work each transfer; K/V projections
    accumulate per contraction chunk as xT arrives; warmup matmuls keep
    the PE clock ramped during the DMA head.
  - V is transposed back to natural [w, dk] layout; each vt tile is
    [ones | junk | v] so the PV matmul emits the softmax denominator in
    PSUM row 0 (readable at partition base 0 by the fast reciprocal)
    and the attention rows at partitions 64:128.
  - Scores S^T[w, q] cover exactly the band windows (128..384 q cols per
    kv chunk), packed with zero waste into one [128, 3, 512] PSUM tile
    per head; ONE batched exp per head (Scalar) writes bf16 pt.
  - Band masking via 8 in-place affine_selects on GpSimd (zero-fill of
    the out-of-band triangles), emitted in PV consumption order.
  - Normalization: reciprocal_approx_fast off PSUM row 0, a one-row
    rounding copy into the f32r selector operand, one 33-contraction
    broadcast matmul and one DVE multiply per pair.
  - psum->attnT casts alternate Scalar (aligned) / DVE (cross-base).
  - Output projection (bf16) starts for two blocks during the pipeline
    drain; y is stored bf16 and upcast on host.
"""

import numpy as np

DIM = 1024
NH = 16  # query heads
G = 2  # kv heads
HD = 64  # head dim
W = 256  # window
HALF = 128
BATCH, SEQ = 2, 2048
NCORES = 8
SQ = 512  # query rows per core
SK = SQ + 2 * HALF  # 768 kv halo rows per core
KC = DIM // 128  # 8 contraction chunks
NJ = SK // 128  # 6 kv chunks

# true band windows [lo, hi) in local q coords per kv chunk; widths
# 128+256+384+384+256+128 = 1536 pack exactly into three 512-col banks
PVW = {0: (0, 128), 1: (0, 256), 2: (0, 384), 3: (128, 512), 4: (256, 512), 5: (384, 512)}
PACK = {2: (0, 0), 0: (0, 384), 3: (1, 0), 5: (1, 384), 1: (2, 0), 4: (2, 256)}  # (slot, col0)
# PV issue order: j1 [0,256) and j4 [256,512) partition the PSUM zero
# region exactly, so every byte is written once before any accumulation
PV_ORDER = [1, 4, 0, 2, 3, 5]

_CACHE = {}


def _build_program(dbg=False):
    import concourse.bass as bass
    import concourse.mybir as mybir
    import concourse.tile as tile
    from concourse import bacc

    f32 = mybir.dt.float32
    f32r = mybir.dt.float32r
    bf16 = mybir.dt.bfloat16

    nc = bacc.Bacc("TRN2", target_bir_lowering=False, debug=False, num_devices=NCORES)
    dbg_t = {}
    if dbg:
        for name, shape, dt in [
            ("dbg_qT", [128, KC, SQ], f32),
            ("dbg_kT", [128, SK], f32),
            ("dbg_vt", [128, NJ, G, 128], f32),
            ("dbg_pt0", [128, 3, 512], f32),
            ("dbg_den", [128, 2, SQ], f32),
            ("dbg_attnT", [128, KC, SQ], f32),
        ]:
            dbg_t[name] = nc.declare_dram_parameter(name, shape, dt, isOutput=True)

    xaT = nc.declare_dram_parameter("xaT", [DIM + 1, SK], bf16, isOutput=False)
    wqb = nc.declare_dram_parameter("wqb", [KC, 128, KC, 128], bf16, isOutput=False)
    wk = nc.declare_dram_parameter("wk", [DIM + 1, G * HD], bf16, isOutput=False)
    wv = nc.declare_dram_parameter("wv", [DIM + 1, G * HD], bf16, isOutput=False)
    wo = nc.declare_dram_parameter("wo", [DIM, DIM], bf16, isOutput=False)
    bq = nc.declare_dram_parameter("bq", [DIM, 1], f32, isOutput=False)
    bo = nc.declare_dram_parameter("bo", [DIM, 1], f32, isOutput=False)
    sel33 = nc.declare_dram_parameter("sel33", [128, 128], f32r, isOutput=False)
    identD = nc.declare_dram_parameter("ident", [128, 128], bf16, isOutput=False)
    ones2 = nc.declare_dram_parameter("ones2", [128, G], bf16, isOutput=False)
    yT = nc.declare_dram_parameter("yT", [DIM, SQ], bf16, isOutput=True)

    with tile.TileContext(nc) as tc:
        with (
            nc.allow_low_precision("bf16/fp32r matmul inputs; accumulation stays fp32"),
            tc.tile_pool(name="wts", bufs=1) as wts,
            tc.tile_pool(name="sb", bufs=1) as sb,
            tc.tile_pool(name="pt", bufs=3) as ptp,
            tc.tile_pool(name="yst", bufs=2) as yst,
            tc.tile_pool(name="psc", bufs=2, space="PSUM") as pscp,
            tc.tile_pool(name="pvP", bufs=2, space="PSUM") as pvP,
        ):
            # ---- small constants ride the GPSIMD SWDGE queue ----
            ident = wts.tile([128, 128], bf16, tag="ident")
            nc.gpsimd.dma_start(out=ident[:, :], in_=identD[:, :])
            sel_sb = wts.tile([128, 128], f32r, tag="sel33")
            nc.gpsimd.dma_start(out=sel_sb[:, :], in_=sel33[:, :])
            ones_sb = wts.tile([128, G], bf16, tag="ones")
            nc.gpsimd.dma_start(out=ones_sb[:, :], in_=ones2[:, :])
            xaug = wts.tile([1, SK], bf16, tag="xaug")
            nc.gpsimd.dma_start(out=xaug[:, :], in_=xaT[DIM:DIM + 1, :])
            wk_aug = wts.tile([1, G * HD], bf16, tag="wkaug")
            wv_aug = wts.tile([1, G * HD], bf16, tag="wvaug")
            nc.gpsimd.dma_start(out=wk_aug[:, :], in_=wk[DIM:DIM + 1, :])
            nc.gpsimd.dma_start(out=wv_aug[:, :], in_=wv[DIM:DIM + 1, :])
            bq_sb = wts.tile([128, KC], f32, tag="bq")
            bo_sb = wts.tile([128, KC], f32, tag="bo")
            nc.gpsimd.dma_start(
                out=bq_sb[:, :], in_=bq.rearrange("(a p) c -> p (a c)", p=128))
            nc.gpsimd.dma_start(
                out=bo_sb[:, :], in_=bo.rearrange("(a p) c -> p (a c)", p=128))

            # ---- big loads: partition-halved across the two HWDGE rings ----
            wk_sb = wts.tile([128, KC, G * HD], bf16, tag="wk")
            wv_sb = wts.tile([128, KC, G * HD], bf16, tag="wv")
            xT_sb = wts.tile([128, KC, SK], bf16, tag="xT")
            wq_sb = wts.tile([128, KC, KC, 128], bf16, tag="wq")
            wkr = wk[0:DIM, :].rearrange("(a p) c -> p a c", p=128)
            wvr = wv[0:DIM, :].rearrange("(a p) c -> p a c", p=128)
            for q4 in range(2):
                a, b = q4 * 32, 64 + q4 * 32
                nc.sync.dma_start(out=wk_sb[a:a + 32, :, :], in_=wkr[a:a + 32])
                nc.scalar.dma_start(out=wk_sb[b:b + 32, :, :], in_=wkr[b:b + 32])
            for q4 in range(2):
                a, b = q4 * 32, 64 + q4 * 32
                nc.sync.dma_start(out=wv_sb[a:a + 32, :, :], in_=wvr[a:a + 32])
                nc.scalar.dma_start(out=wv_sb[b:b + 32, :, :], in_=wvr[b:b + 32])
            for kc in range(KC):
                r0 = kc * 128
                if kc < 3:  # earliest chunks: 4-way split for low latency
                    for q4 in range(2):
                        a = r0 + q4 * 32
                        b = r0 + 64 + q4 * 32
                        nc.sync.dma_start(out=xT_sb[q4 * 32:q4 * 32 + 32, kc, :],
                                          in_=xaT[a:a + 32, :])
                        nc.scalar.dma_start(
                            out=xT_sb[64 + q4 * 32:96 + q4 * 32, kc, :],
                            in_=xaT[b:b + 32, :])
                else:
                    nc.sync.dma_start(out=xT_sb[0:64, kc, :], in_=xaT[r0:r0 + 64, :])
                    nc.scalar.dma_start(out=xT_sb[64:128, kc, :],
                                        in_=xaT[r0 + 64:r0 + 128, :])
                if kc == 1:
                    nc.sync.dma_start(out=wq_sb[0:64, 0, :, :], in_=wqb[0, 0:64])
                    nc.scalar.dma_start(out=wq_sb[64:128, 0, :, :],
                                        in_=wqb[0, 64:128])
            for dd in range(1, KC):
                nc.sync.dma_start(out=wq_sb[0:64, dd, :, :], in_=wqb[dd, 0:64])
                nc.scalar.dma_start(out=wq_sb[64:128, dd, :, :], in_=wqb[dd, 64:128])
            wo_sb = wts.tile([128, KC, DIM], bf16, tag="wo")
            for kc in range(KC):
                eng = nc.sync if kc % 2 == 0 else nc.scalar
                eng.dma_start(out=wo_sb[:, kc, :], in_=wo[kc * 128:(kc + 1) * 128, :])

            # ---- persistent intermediates ----
            qT_sb = sb.tile([128, KC, SQ], bf16, tag="qT")   # [dk(2 heads), dd, q]
            kT_sb = sb.tile([128, SK], bf16, tag="kT")       # [dk(2 groups), w]
            vT_sb = sb.tile([128, SK], bf16, tag="vT")
            # vt[j] = [ones | junk(63) | v(64)] so PV emits den at psum row 0
            # and attention rows at partitions 64:128
            vt_t = [
                sb.tile([128, G, 128], bf16, tag=f"vt{j}", name=f"vt{j}")
                for j in range(NJ)
            ]
            attnT = sb.tile([128, KC, SQ], bf16, tag="attnT")  # [dk(2 heads), pair, q]
            # reciprocal denominators: row 0 = head p, row 32 = head p+8;
            # rows 1..31 filled 1.0 once so sel33 contracts finite values
            den_r2 = sb.tile([128, 2, SQ], f32r, tag="denr2")
            nc.gpsimd.affine_select(
                out=den_r2[:, :, :], in_=den_r2[:, :, :],
                compare_op=mybir.AluOpType.is_ge, fill=1.0,
                base=-1 << 20, channel_multiplier=1,
                pattern=[[1, 2], [1, SQ]],
            )

            # ---- K/V projections, chunk-accumulated as the xT DMA lands ----
            pscK = pscp.tile([128, 3, 512], f32, tag="psc", name="pscK")
            pscV = pscp.tile([128, 3, 512], f32, tag="psc", name="pscV")
            ndum = 0
            for kc in range(KC):
                for h2 in range(2):
                    sl = slice(h2 * 384, (h2 + 1) * 384)
                    nc.tensor.matmul(
                        pscK[:, h2, 0:384], wk_sb[:, kc, :], xT_sb[:, kc, sl],
                        start=(kc == 0), stop=False,
                    )
                    nc.tensor.matmul(
                        pscV[:, h2, 0:384], wv_sb[:, kc, :], xT_sb[:, kc, sl],
                        start=(kc == 0), stop=False,
                    )
                if kc >= 1:
                    dum = pvP.tile([128, 512], f32, tag="pv", name=f"dum{ndum}")
                    ndum += 1
                    nc.tensor.matmul(dum[:, :], ident[:, :],
                                     xT_sb[:, kc, 0:512], start=True, stop=True)
            for h2 in range(2):
                sl = slice(h2 * 384, (h2 + 1) * 384)
                nc.tensor.matmul(pscK[:, h2, 0:384], wk_aug[:, :], xaug[:, sl],
                                 start=False, stop=True)
                nc.tensor.matmul(pscV[:, h2, 0:384], wv_aug[:, :], xaug[:, sl],
                                 start=False, stop=True)
            for h2 in range(2):
                sl = slice(h2 * 384, (h2 + 1) * 384)
                nc.vector.tensor_copy(kT_sb[:, sl], pscK[:, h2, 0:384])
                nc.vector.tensor_copy(vT_sb[:, sl], pscV[:, h2, 0:384])

            # ---- V back to natural layout [w, dk] ----
            for j in range(NJ):
                ps = pvP.tile([128, 512], bf16, tag="pv", name=f"pstr{j}")
                out = ps[:, 0:128]
                nc.tensor.transpose(out, vT_sb[:, j * 128:(j + 1) * 128], ident)
                nc.vector.tensor_copy(
                    vt_t[j][:, :, 64:128],
                    out.rearrange("p (g d) -> p g d", g=G),
                )
                nc.vector.tensor_copy(vt_t[j][:, :, 0:1], ones_sb[:, :])

            def q_proj(dd):
                ps = pvP.tile([128, 512], f32, tag="pv", name=f"psq{dd}")
                for kc in range(KC):
                    nc.tensor.matmul(
                        ps[:, :], wq_sb[:, dd, kc, :],
                        xT_sb[:, kc, HALF:HALF + SQ],
                        start=(kc == 0), stop=(kc == KC - 1),
                    )
                nc.scalar.activation(
                    qT_sb[:, dd, :], ps[:, :], mybir.ActivationFunctionType.Identity,
                    bias=bq_sb[:, dd:dd + 1],
                )

            if dbg:
                nc.sync.dma_start(out=dbg_t["dbg_kT"][:, :], in_=kT_sb[:, :])
                for j in range(NJ):
                    nc.sync.dma_start(out=dbg_t["dbg_vt"][:, j, :, :], in_=vt_t[j][:, :, :])

            # masking selects, merged: keep iff 0 <= 128j + ww - q < 256 with
            # q = PVW[j][0] + c, restricted to the genuinely dirty columns.
            # Selects with identical affine params merge across psc slots via
            # a stride-0 (or strided) slot dimension in one AP:
            #   j1-up:    slot2 [129:256)            base -1 chm +1
            #   j4+j5-lo: slot2[256:384)+slot1[384:512) (stride 384 twist;
            #             but slot1 precedes slot2, so express from slot 1)
            #   j0-up:    slot0 [384:512)            base 0  chm +1
            #   j2+j3-up: slots0,1 [257:384)         base -1 chm +1
            #   j2+j3-lo: slots0,1 [0:128)           base -1 chm -1
            def emit_masks(pt):
                nc.gpsimd.affine_select(  # j1 upper
                    out=pt[:, 2, 129:256], in_=pt[:, 2, 129:256],
                    compare_op=mybir.AluOpType.is_ge, fill=0.0,
                    base=-1, channel_multiplier=1,
                    pattern=[[-1, 127]],
                )
                nc.gpsimd.affine_select(  # j0 upper
                    out=pt[:, 0, 384:512], in_=pt[:, 0, 384:512],
                    compare_op=mybir.AluOpType.is_ge, fill=0.0,
                    base=0, channel_multiplier=1,
                    pattern=[[-1, 128]],
                )
                nc.gpsimd.affine_select(  # j2+j3 upper
                    out=pt[:, 0:2, 257:384], in_=pt[:, 0:2, 257:384],
                    compare_op=mybir.AluOpType.is_ge, fill=0.0,
                    base=-1, channel_multiplier=1,
                    pattern=[[0, 2], [-1, 127]],
                )
                nc.gpsimd.affine_select(  # j2+j3 lower
                    out=pt[:, 0:2, 0:128], in_=pt[:, 0:2, 0:128],
                    compare_op=mybir.AluOpType.is_ge, fill=0.0,
                    base=-1, channel_multiplier=-1,
                    pattern=[[0, 2], [1, 128]],
                )
                # j5 lower (slot1 cols 384:512) + j4 lower (slot2 cols
                # 256:384): flat offsets 896 and 1280 = stride-384 pair; both
                # share base -1 / chm -1, so one select covers both regions
                ptf = pt.rearrange("p a b -> p (a b)")[:, 896:1408].rearrange(
                    "p (a b) -> p a b", a=4)[:, 0::3, :]
                nc.gpsimd.affine_select(
                    out=ptf, in_=ptf,
                    compare_op=mybir.AluOpType.is_ge, fill=0.0,
                    base=-1, channel_multiplier=-1,
                    pattern=[[0, 2], [1, 128]],
                )

            # ---- attention: software-pipelined head loop ----
            order = [(p, gg) for p in range(KC) for gg in range(G)]
            pt_t = {}
            oproj_ps = {}
            q_proj(0)
            for i in range(len(order) + 4):
                if i < len(order):
                    p, gg = order[i]
                    h = p + 8 * gg
                    g = gg
                    qT_h = qT_sb[64 * gg:64 * gg + 64, p, :]
                    psc = pscp.tile([128, 3, 512], f32, tag="psc", name=f"psc{h}")
                    for j in PV_ORDER:
                        slot, c0 = PACK[j]
                        slo, shi = PVW[j]
                        nc.tensor.matmul(
                            psc[:, slot, c0:c0 + (shi - slo)],
                            kT_sb[64 * g:64 * g + 64, j * 128:(j + 1) * 128],
                            qT_h[:, slo:shi],
                            start=True, stop=True,
                        )
                    pt = ptp.tile([128, 3, 512], bf16, tag="pt", name=f"pt{h}")
                    pt_t[i] = pt
                    nc.scalar.activation(pt[:, :, :], psc[:, :, :],
                                         mybir.ActivationFunctionType.Exp)
                    emit_masks(pt)
                    if gg == 0 and p < KC - 1:
                        q_proj(p + 1)
                # PV + norm chain for head i-2 (after this head's scores/exp issue)
                if i >= 2 and i - 2 < len(order):
                    p2, g2 = order[i - 2]
                    h2 = p2 + 8 * g2
                    pt2 = pt_t[i - 2]
                    pv = pvP.tile([128, 512], f32, tag="pv", name=f"pv{h2}")
                    for j in PV_ORDER:
                        lo, hi = PVW[j]
                        slot, c0 = PACK[j]
                        nc.tensor.matmul(
                            pv[:, lo:hi],
                            vt_t[j][:, g2, :],
                            pt2[:, slot, c0:c0 + (hi - lo)],
                            start=(j == PV_ORDER[0]), stop=(j == PV_ORDER[-1]),
                        )
                    if g2 == 0:
                        nc.vector.tensor_copy(attnT[0:64, p2, :], pv[64:128, :])
                    else:
                        nc.scalar.activation(attnT[64:128, p2, :], pv[64:128, :],
                                             mybir.ActivationFunctionType.Copy)
                    s2 = p2 % 2
                    drh = yst.tile([1, SQ], f32, tag="drh", name=f"drh{h2}")
                    nc.vector.reciprocal_approx_fast(out=drh[:, :], in_=pv[0:1, :])
                    nc.vector.tensor_copy(den_r2[32 * g2:32 * g2 + 1, s2, :],
                                          drh[:, :])
                if i >= 5 and (i - 5) % 2 == 0 and (i - 5) // 2 < KC:
                    pr = (i - 5) // 2
                    rb = pvP.tile([128, 512], f32, tag="pv", name=f"rb{pr}")
                    nc.tensor.matmul(rb[:, :], sel_sb[0:33, :],
                                     den_r2[0:33, pr % 2, :], start=True, stop=True)
                    nc.vector.tensor_mul(attnT[:, pr, :], attnT[:, pr, :], rb[:, :])
                if i in (len(order) + 1, len(order) + 2, len(order) + 3):
                    # drain window: prefill O-proj do=0..3 over pairs 0..6
                    # (do 0/1 on psc banks, do 2/3 on pv banks after rb7)
                    if i == len(order) + 1:
                        dos = [(0, pscp, "psc", [128, 3, 512])]
                    elif i == len(order) + 2:
                        dos = [(1, pscp, "psc", [128, 3, 512])]
                    else:  # after rb7 (emitted at i = len(order)+3)
                        dos = [(2, pvP, "pv", [128, 512]), (3, pvP, "pv", [128, 512])]
                    for do, pool, tag, shp in dos:
                        ps = pool.tile(shp, f32, tag=tag, name=f"pso{do}")
                        oproj_ps[do] = ps
                        out_ap = ps[:, 0, :] if len(shp) == 3 else ps[:, :]
                        for p in range(KC - 1):
                            nc.tensor.matmul(
                                out_ap, wo_sb[:, p, do * 128:(do + 1) * 128],
                                attnT[:, p, :],
                                start=(p == 0), stop=False,
                            )

            if dbg:
                nc.sync.dma_start(out=dbg_t["dbg_qT"][:, :, :], in_=qT_sb[:, :, :])
                nc.sync.dma_start(out=dbg_t["dbg_pt0"][:, :, :], in_=pt_t[0][:, :, :])
                nc.sync.dma_start(out=dbg_t["dbg_den"][:, :, :], in_=den_r2[:, :, :])
                nc.sync.dma_start(out=dbg_t["dbg_attnT"][:, :, :], in_=attnT[:, :, :])

            # ---- output projection ----
            for do in range(KC):
                if do < 4:
                    ps = oproj_ps[do]
                    out_ap = ps[:, 0, :] if do < 2 else ps[:, :]
                else:
                    ps = pscp.tile([128, 3, 512], f32, tag="psc", name=f"pso{do}")
                    out_ap = ps[:, 0, :]
                for p in range(0 if do >= 4 else KC - 1, KC):
                    nc.tensor.matmul(
                        out_ap, wo_sb[:, p, do * 128:(do + 1) * 128],
                        attnT[:, p, :],
                        start=(p == 0), stop=(p == KC - 1),
                    )
                yt = yst.tile([128, SQ], bf16, tag="yt")
                nc.scalar.activation(yt, out_ap,
                                     mybir.ActivationFunctionType.Identity,
                                     bias=bo_sb[:, do:do + 1])
                r0 = do * 128
                nc.sync.dma_start(out=yT[r0:r0 + 64, :], in_=yt[0:64, :])
                nc.scalar.dma_start(out=yT[r0 + 64:r0 + 128, :], in_=yt[64:128, :])

    nc.finalize()
    return nc


def get_program(dbg=False):
    key = ("nc", dbg)
    if key not in _CACHE:
        _CACHE[key] = _build_program(dbg)
    return _CACHE[key]


def make_in_maps(x, Wq, bq, Wk, bk, Wv, bv, Wo, bo):
    """Host-side sharding: per-core input dicts."""
    import ml_dtypes

    bft = ml_dtypes.bfloat16
    x = np.ascontiguousarray(np.asarray(x, np.float32))
    wkb = np.concatenate([np.asarray(Wk, np.float32), np.asarray(bk, np.float32)[None]], 0)
    wvb = np.concatenate([np.asarray(Wv, np.float32), np.asarray(bv, np.float32)[None]], 0)
    sel33 = np.zeros((128, 128), np.float32)
    sel33[0, 0:64] = 1.0
    sel33[32, 64:128] = 1.0
    # head permutation: device column-block p holds [head p | head p+8]
    perm = np.empty(DIM, np.int64)
    for p in range(8):
        perm[128 * p:128 * p + 64] = np.arange(64 * p, 64 * p + 64)
        perm[128 * p + 64:128 * p + 128] = np.arange(64 * (p + 8), 64 * (p + 8) + 64)
    wqp = np.asarray(Wq, np.float32)[:, perm]
    # wqb[dd, p, kc, c] = wqp[kc*128+p, dd*128+c]: dd-block-major layout
    wqb = np.ascontiguousarray(
        wqp.reshape(KC, 128, KC, 128).transpose(2, 1, 0, 3))
    common = {
        "wqb": wqb.astype(bft),
        "wk": np.ascontiguousarray(wkb).astype(bft),
        "wv": np.ascontiguousarray(wvb).astype(bft),
        "wo": np.ascontiguousarray(np.asarray(Wo, np.float32)[perm, :]).astype(bft),
        "bq": np.ascontiguousarray(np.asarray(bq, np.float32)[perm].reshape(DIM, 1)),
        "bo": np.ascontiguousarray(np.asarray(bo, np.float32).reshape(DIM, 1)),
        "sel33": sel33,
        "ident": np.eye(128, dtype=np.float32).astype(bft),
        "ones2": np.ones((128, G), bft),
    }
    in_maps = []
    for c in range(NCORES):
        b, t = divmod(c, NCORES // BATCH)
        s0 = SQ * t
        xa = np.zeros((SK, DIM + 1), np.float32)
        lo, hi = max(0, s0 - HALF), min(SEQ, s0 + SQ + HALF)
        xa[lo - (s0 - HALF):hi - (s0 - HALF), :DIM] = x[b, lo:hi]
        xa[lo - (s0 - HALF):hi - (s0 - HALF), DIM] = 1.0
        in_maps.append({"xaT": np.ascontiguousarray(xa.T).astype(bft), **common})
    return in_maps


def assemble_output(results):
    y = np.empty((BATCH, SEQ, DIM), np.float32)
    for c in range(NCORES):
        b, t = divmod(c, NCORES // BATCH)
        y[b, SQ * t:SQ * (t + 1), :] = np.asarray(results[c]["yT"], np.float32).T
    return y


def kernel(**inputs):
    from concourse.bass_utils import run_bass_kernel_spmd

    nc = get_program()
    in_maps = make_in_maps(**inputs)
    last_err = None
    for _ in range(3):  # retry: transient NRT device wedges recover on rerun
        try:
            res = run_bass_kernel_spmd(nc, in_maps, list(range(NCORES)))
            return assemble_output(res.results)
        except Exception as e:  # noqa: BLE001
            last_err = e
    raise last_err
